# revision 38
# baseline (speedup 1.0000x reference)
"""Trainium2 Bass kernel for the triplet exp-distance loss.

loss = mean_i[ D_ap*(D_ap - v_ap)^2 + D_an*(D_an - v_an)^2 ]
  D_xx = exp(-triplets_dis[batch_index][:, k])
  v_xx = exp(-||a - x||_2)

Strategy: pure data parallel over 8 NeuronCores (65536 rows each),
scalar partials reduced on the host.

Dataflow (default CFG, selected by TimelineSim sweeps):
- Embeddings are cast to bf16 on the host before upload, halving the
  HBM/DMA roofline vs f32 (host-simulated rel err ~2e-6, gate 2e-2).
- p and n are uploaded negated so the subtractions ride the DMA:
  per 32-row chunk, `a` loads once (HWDGE), a DVE 4x tensor_copy
  clones it, and SWDGE accum-DMAs add -p / -n into the clones while
  streaming them in (the DMA engines' inline CCE adders do the
  subtract for free; emitted in 4KB-per-partition pieces — larger
  accum transfers fail at runtime on this toolchain).
- Squares run in place, split ~3:1 between ACT and DVE (2x mul);
  per-row squared norms come from a bf16 2x binary add-tree to width
  8 plus an f32 reduce.  Pair p's tree level 1 runs on GPSIMD, which
  otherwise only does SWDGE descriptor generation, so no engine
  exceeds ~80% of the DMA period.
- Tail groups (sqrt/exp on ACT, weighted error on GPSIMD, reduce on
  DVE into [128, ntail, 2] partials) are emitted one chunk after
  their rows complete so the in-order engine queues never head-of-line
  block on them.
- The stream ends with four 8-row mini chunks using independent
  HWDGE loads + DVE adds (no accum chain), and the last `end_span`
  chunks route everything through short DVE/ACT paths, keeping the
  post-DMA dependency chain and pipeline drain small.
"""

import numpy as np
import ml_dtypes

import concourse.bass as bass
import concourse.mybir as mb
import concourse.tile as tile
from concourse.bass_utils import run_bass_kernel_spmd

B = 524288
D = 128
M = 8                 # cores
S = B // M            # rows per core = 65536
P = 128               # SBUF partitions
RPP = S // P          # rows per partition = 512

F32 = mb.dt.float32
BF16 = mb.dt.bfloat16
NP_BF16 = ml_dtypes.bfloat16

# --- tunable configuration (TimelineSim-swept) -------------------------
CFG = {
    "mode": "accum2",   # "sub2": 2 DVE subs; "accum2": copy + 2 accum DMAs;
                        # "hybrid": 1 DVE sub (pair n) + 1 accum DMA (pair p)
    "cbig": 32,         # rows/partition per big chunk
    "nmini": 4,         # trailing mini chunks
    "cmini": 8,         # rows/partition per mini chunk
    "io_bufs": 3,
    "wk_bufs": 2,
    "pool_h1": "p",     # which pairs' first tree level runs on GPSIMD: "", "p", "pn"
    "pool_tail": True,  # tail t/m/sc on GPSIMD instead of DVE
    "tail_delay": 1,    # chunks between data-ready and tail emission
    "iom_bufs": 2,
    "wkm_bufs": 2,
    "end_span": 7,      # last chunks forced onto the short-chain path
    "tl_bufs": 2,
    "sub2_span": 0,     # trailing chunks that use independent loads + DVE adds
    "mini_pool_h1": "", # mini-chunk pairs whose tree level 1 runs on GPSIMD
    "interleave": False,  # interleave the two pairs' tree stages
    "split_copy": False,
    "tail_rows": 64,   # rows per tail group flush threshold
    "h1_no_pool_on_tail": False,
    "h1_split": False,
    "end_halves": 0,    # trailing big chunks computed per 16-row accum piece
    # columns (of D=128) squared on ACT per (big-chunk, pair); the rest
    # squares on DVE as a 2x tensor_mul
    "sq_act_cols": {(True, "p"): 96, (True, "n"): 96,
                    (False, "p"): 64, (False, "n"): 64},
}


def _split_multiwaits(nc):
    """This walrus build accepts only one sync-wait per instruction.
    Hoist extra waits onto standalone single-wait InstEventSemaphore
    instructions inserted just before, on the same engine (semantically
    identical: the engine queue blocks on each in sequence)."""
    n_split = 0
    for f in nc.m.functions:
        for bb in f.blocks:
            insts = bb.instructions
            out = []
            changed = False
            for ins in insts:
                si = getattr(ins, "sync_info", None)
                if si is not None and si.on_wait is not None and len(si.on_wait) > 1:
                    waits = list(si.on_wait)
                    for k, w in enumerate(waits[:-1]):
                        ev = mb.InstEventSemaphore(
                            name=f"{ins.name}-wsplit{k}",
                            engine=ins.engine,
                            ins=[],
                            outs=[],
                            sync_info=mb.SyncInfo(on_wait=[w], on_update=[]),
                        )
                        out.append(ev)
                        n_split += 1
                    si.on_wait.clear()
                    si.on_wait.append(waits[-1])
                    changed = True
                out.append(ins)
            if changed:
                bb.instructions = out
    return n_split


def _build(cfg=None):
    cfg = dict(CFG, **(cfg or {}))
    mode = cfg["mode"]
    if cfg.get("sched"):
        sizes = list(cfg["sched"])
    else:
        CBIG, CMINI, NMINI = cfg["cbig"], cfg["cmini"], cfg["nmini"]
        NBIG = (RPP - NMINI * CMINI) // CBIG
        assert NBIG * CBIG + NMINI * CMINI == RPP
        sizes = [CBIG] * NBIG + [CMINI] * NMINI
    assert sum(sizes) == RPP
    chunks = []
    row = 0
    for C in sizes:
        chunks.append((row, C, C >= 32))   # <32-row chunks take the mini path
        row += C
    # tail groups: flush a group whenever >=64 rows have accumulated, plus a
    # final remainder group
    tails = []
    g_rows = 0
    for idx, (row0, C, big) in enumerate(chunks):
        done = row0 + C
        last = idx == len(chunks) - 1
        if done - g_rows >= cfg["tail_rows"] or (last and done > g_rows):
            if last and idx >= 1 and done - g_rows > 32 and done - g_rows <= 64 \
               and chunks[idx - 1][0] + chunks[idx - 1][1] - g_rows >= 32:
                # split the final stretch so the very last group stays small
                mid = chunks[idx - 1][0] + chunks[idx - 1][1]
                tails.append((idx - 1, g_rows, mid - g_rows))
                g_rows = mid
            tails.append((idx, g_rows, done - g_rows))
            g_rows = done
    ntail = len(tails)

    nc = bass.Bass(trn_type="TRN2", name="triplet_loss")
    a = nc.dram_tensor("a", [S, D], BF16, kind="ExternalInput")
    p = nc.dram_tensor("p", [S, D], BF16, kind="ExternalInput")   # holds -p
    n = nc.dram_tensor("n", [S, D], BF16, kind="ExternalInput")   # -n (accum modes)
    td = nc.dram_tensor("td", [S, 2], BF16, kind="ExternalInput")
    out = nc.dram_tensor("out", [P, ntail * 2], F32, kind="ExternalOutput")

    av = a.rearrange("(p n) d -> p (n d)", p=P)    # [128, RPP*D]
    pv = p.rearrange("(p n) d -> p (n d)", p=P)
    nv = n.rearrange("(p n) d -> p (n d)", p=P)
    tdv = td.rearrange("(p n) t -> p n t", p=P)    # [128, RPP, 2]

    with tile.TileContext(nc) as tc:
        with tc.tile_pool(name="io", bufs=cfg["io_bufs"]) as io, \
             tc.tile_pool(name="iom", bufs=cfg["iom_bufs"]) as iom, \
             tc.tile_pool(name="wk", bufs=cfg["wk_bufs"]) as wk, \
             tc.tile_pool(name="wkm", bufs=cfg["wkm_bufs"]) as wkm, \
             tc.tile_pool(name="tl", bufs=cfg["tl_bufs"]) as tl, \
             tc.tile_pool(name="res", bufs=1) as res:
            td_t = res.tile([P, RPP, 2], BF16)
            dex = res.tile([P, RPP, 2], F32)

            n2 = {}
            n2["p"] = res.tile([P, RPP], F32, tag="n2p", name="n2p")
            n2["n"] = res.tile([P, RPP], F32, tag="n2n", name="n2n")
            acc = res.tile([P, ntail, 2], F32)   # [P, tail group, pair]

            def emit_dmas(ci):
                """Produce the two difference tiles (tp = +-(a-p),
                tn = +-(a-n)) for chunk ci, per cfg['mode']."""
                row0, C, big = chunks[ci]
                FD = C * D
                sl = slice(row0 * D, (row0 + C) * D)
                pool = io if big else iom
                sfx = "" if big else "m"
                ta = pool.tile([P, FD], BF16, tag="a" + sfx)
                nc.sync.dma_start(out=ta, in_=av[:, sl])
                if not big or ci >= len(chunks) - cfg["sub2_span"]:
                    # mini chunks close out the stream with three independent
                    # HWDGE loads + DVE adds: no accum chain, so the last
                    # transfers land back-to-back and the post-DMA chain is
                    # short.
                    tp_r = pool.tile([P, FD], BF16, tag="p" + sfx)
                    nc.sync.dma_start(out=tp_r, in_=pv[:, sl])
                    tn_r = pool.tile([P, FD], BF16, tag="n" + sfx)
                    nc.sync.dma_start(out=tn_r, in_=nv[:, sl])
                    tp = wkm.tile([P, FD], BF16, tag="dp" + sfx)
                    nc.vector.tensor_add(out=tp, in0=ta, in1=tp_r)
                    tn = wkm.tile([P, FD], BF16, tag="dn" + sfx)
                    nc.vector.tensor_add(out=tn, in0=ta, in1=tn_r)
                    return tp, tn
                if mode == "accum2":
                    tn = pool.tile([P, FD], BF16, tag="n" + sfx)
                    if cfg["split_copy"]:
                        Hc = FD // 2
                        nc.vector.tensor_copy(out=tn[:, 0:Hc], in_=ta[:, 0:Hc])
                        nc.vector.tensor_copy(out=tn[:, Hc:FD], in_=ta[:, Hc:FD])
                    else:
                        nc.vector.tensor_copy(out=tn, in_=ta)
                    # SWDGE accum DMAs fail at runtime above 4KB/partition:
                    # emit in <=2048-element pieces.
                    H = min(FD, 2048)
                    for h0 in range(0, FD, H):
                        nc.gpsimd.dma_start(
                            out=ta[:, h0 : h0 + H],
                            in_=pv[:, row0 * D + h0 : row0 * D + h0 + H],
                            accum_op=mb.AluOpType.add)
                    for h0 in range(0, FD, H):
                        nc.gpsimd.dma_start(
                            out=tn[:, h0 : h0 + H],
                            in_=nv[:, row0 * D + h0 : row0 * D + h0 + H],
                            accum_op=mb.AluOpType.add)
                    return ta, tn
                elif mode == "hybrid":
                    tr = pool.tile([P, FD], BF16, tag="n" + sfx)
                    nc.sync.dma_start(out=tr, in_=nv[:, sl])
                    tn = pool.tile([P, FD], BF16, tag="dn" + sfx)
                    nc.vector.tensor_add(out=tn, in0=ta, in1=tr)  # a + (-n)
                    nc.gpsimd.dma_start(out=ta, in_=pv[:, sl],
                                        accum_op=mb.AluOpType.add)
                    return ta, tn
                else:  # sub2
                    tp_r = pool.tile([P, FD], BF16, tag="p" + sfx)
                    nc.sync.dma_start(out=tp_r, in_=pv[:, sl])
                    tn_r = pool.tile([P, FD], BF16, tag="n" + sfx)
                    nc.sync.dma_start(out=tn_r, in_=nv[:, sl])
                    tp = wk.tile([P, FD], BF16, tag="dp" + sfx)
                    nc.vector.tensor_add(out=tp, in0=ta, in1=tp_r)
                    tn = wk.tile([P, FD], BF16, tag="dn" + sfx)
                    nc.vector.tensor_add(out=tn, in0=ta, in1=tn_r)
                    return tp, tn

            def emit_compute(ci, tp, tn, rows=None):
                row0, C, big = chunks[ci]
                if rows is not None:
                    # piece-wise compute: consume a row-range matching one
                    # 4KB accum DMA piece so the end chain stays short
                    row0, C = row0 + rows[0], rows[1] - rows[0]
                pool = wk if big else wkm
                sfx = "" if big else "m"
                # the last end_span chunks take the short-chain path (no Pool
                # h1, balanced sq split) — their latency is exposed at the end
                hot_end = ci >= len(chunks) - cfg["end_span"]
                stages = {}
                for key, ot in (("p", tp), ("n", tn)):
                    ot3 = ot.rearrange("p (c d) -> p c d", d=D)
                    if rows is not None:
                        ot3 = ot3[:, rows[0] : rows[1], :]
                    stages[key] = ot3
                def emit_pair(key, stage):
                    ot3 = stages[key]
                    if stage == 0:
                        # square in place; part on DVE (2x mul) to keep ACT
                        # off the critical path
                        q = cfg["sq_act_cols"][(big and not hot_end, key)]
                        if q > 0:
                            nc.scalar.activation(
                                out=ot3[:, :, 0:q], in_=ot3[:, :, 0:q],
                                func=mb.ActivationFunctionType.Square)
                        if q < D:
                            nc.vector.tensor_mul(
                                out=ot3[:, :, q:D], in0=ot3[:, :, q:D],
                                in1=ot3[:, :, q:D])
                        # bf16 2x add-tree level 1
                        h1 = pool.tile([P, C, D // 2], BF16, tag="h1" + key + sfx)
                        use_pool = (big and not hot_end and key in cfg["pool_h1"]) or \
                                   (not big and key in cfg["mini_pool_h1"])
                        if use_pool and cfg["h1_no_pool_on_tail"] and \
                           (ci in tails_after_set):
                            use_pool = False
                        if use_pool and cfg["h1_split"]:
                            Ch = C // 2
                            nc.gpsimd.tensor_add(
                                out=h1[:, 0:Ch, :], in0=ot3[:, 0:Ch, 0 : D // 2],
                                in1=ot3[:, 0:Ch, D // 2 : D])
                            nc.vector.tensor_add(
                                out=h1[:, Ch:C, :], in0=ot3[:, Ch:C, 0 : D // 2],
                                in1=ot3[:, Ch:C, D // 2 : D])
                        else:
                            eng = nc.gpsimd if use_pool else nc.vector
                            eng.tensor_add(
                                out=h1, in0=ot3[:, :, 0 : D // 2], in1=ot3[:, :, D // 2 : D]
                            )
                        stages[key + "h"] = h1
                        return
                    h1 = stages[key + "h"]
                    h2 = pool.tile([P, C, D // 4], BF16, tag="h2" + key + sfx)
                    nc.vector.tensor_add(
                        out=h2, in0=h1[:, :, 0 : D // 4], in1=h1[:, :, D // 4 : D // 2]
                    )
                    h3 = pool.tile([P, C, D // 8], BF16, tag="h3" + key + sfx)
                    nc.vector.tensor_add(
                        out=h3, in0=h2[:, :, 0 : D // 8], in1=h2[:, :, D // 8 : D // 4]
                    )
                    h4 = pool.tile([P, C, D // 16], BF16, tag="h4" + key + sfx)
                    nc.vector.tensor_add(
                        out=h4, in0=h3[:, :, 0 : D // 16], in1=h3[:, :, D // 16 : D // 8]
                    )
                    nc.vector.reduce_sum(
                        out=n2[key][:, row0 : row0 + C],
                        in_=h4,
                        axis=mb.AxisListType.X,
                    )
                if cfg["interleave"]:
                    emit_pair("p", 0); emit_pair("n", 0)
                    emit_pair("p", 1); emit_pair("n", 1)
                else:
                    emit_pair("p", 0); emit_pair("p", 1)
                    emit_pair("n", 0); emit_pair("n", 1)

            def emit_tail(g):
                _, row0, TR = tails[g]
                rs = slice(row0, row0 + TR)
                teng = nc.gpsimd if cfg["pool_tail"] else nc.vector
                for i, key in enumerate(("p", "n")):
                    rt = tl.tile([P, TR], F32, tag="rt" + key)
                    nc.scalar.activation(out=rt, in_=n2[key][:, rs],
                                         func=mb.ActivationFunctionType.Sqrt)
                    nv_ = tl.tile([P, TR], F32, tag="v" + key)
                    nc.scalar.activation(out=nv_, in_=rt,
                                         func=mb.ActivationFunctionType.Exp, scale=-1.0)
                    dcol = dex[:, rs, i]
                    t_ = tl.tile([P, TR], F32, tag="t" + key)
                    teng.tensor_sub(out=t_, in0=dcol, in1=nv_)
                    m_ = tl.tile([P, TR], F32, tag="m" + key)
                    teng.tensor_mul(out=m_, in0=dcol, in1=t_)
                    sc = tl.tile([P, TR], F32, tag="sc" + key)
                    teng.tensor_mul(out=sc, in0=m_, in1=t_)
                    nc.vector.reduce_sum(
                        out=acc[:, g, i : i + 1], in_=sc, axis=mb.AxisListType.X
                    )

            tiles = {}
            tails_after = {after: g for g, (after, _, _) in enumerate(tails)}
            tails_after_set = set(a + cfg["tail_delay"] for a in tails_after)
            tdel = cfg["tail_delay"]
            nch = len(chunks)

            def emit_compute_maybe_halved(ci):
                tp_, tn_ = tiles.pop(ci)
                _, Cc, bigc = chunks[ci]
                if bigc and ci >= nch - cfg["end_halves"]:
                    emit_compute(ci, tp_, tn_, rows=(0, Cc // 2))
                    emit_compute(ci, tp_, tn_, rows=(Cc // 2, Cc))
                else:
                    emit_compute(ci, tp_, tn_)
            for c in range(nch):
                tiles[c] = emit_dmas(c)
                if c == 0:
                    # td load + exp-distances, after chunk 0's loads so the
                    # embedding stream starts immediately
                    nc.sync.dma_start(out=td_t, in_=tdv)
                    nc.scalar.activation(out=dex, in_=td_t,
                                         func=mb.ActivationFunctionType.Exp,
                                         scale=-1.0)
                if c >= 1:
                    emit_compute_maybe_halved(c - 1)
                if c >= 1 + tdel and (c - 1 - tdel) in tails_after:
                    emit_tail(tails_after[c - 1 - tdel])
            emit_compute_maybe_halved(nch - 1)
            for after in sorted(tails_after):
                if after >= nch - 1 - tdel:
                    emit_tail(tails_after[after])

            nc.sync.dma_start(out=out[:, :], in_=acc.rearrange("p g i -> p (g i)"))

    _split_multiwaits(nc)
    nc._ntail = ntail
    return nc


_CACHE = {}


def _get_nc():
    if "nc" not in _CACHE:
        _CACHE["nc"] = _build()
    return _CACHE["nc"]


def _run(inputs, **spmd_kwargs):
    a = np.asarray(inputs["embedding_a"], dtype=np.float32).astype(NP_BF16)
    p = (-np.asarray(inputs["embedding_p"], dtype=np.float32)).astype(NP_BF16)
    n = (-np.asarray(inputs["embedding_n"], dtype=np.float32)).astype(NP_BF16)
    tdis = np.asarray(inputs["triplets_dis"], dtype=np.float32)
    bidx = np.asarray(inputs["batch_index"])
    td = np.ascontiguousarray(tdis[bidx]).astype(NP_BF16)

    in_maps = [
        {
            "a": a[i * S : (i + 1) * S],
            "p": p[i * S : (i + 1) * S],
            "n": n[i * S : (i + 1) * S],
            "td": td[i * S : (i + 1) * S],
        }
        for i in range(M)
    ]
    r = run_bass_kernel_spmd(_get_nc(), in_maps, core_ids=list(range(M)), **spmd_kwargs)
    total = sum(res["out"].astype(np.float64).sum() for res in r.results)
    return np.float32(total / B), r


def kernel(**inputs):
    loss, _ = _run(inputs)
    return loss
